# revision 3
# baseline (speedup 1.0000x reference)
"""LSTM-cell (shared-gate) Trainium2 kernel.

Reference computes, for B=8192, IN=H=4096:
    z = x @ Wi.T + bi + h @ Wh.T + bh        # [B, H]
    s = sigmoid(z); g = tanh(z)
    c_new = c*s + s*g = s*(c+g)
    out = s*tanh(c_new)
    returns (out, c_new)

Strategy: data-parallel over batch across 8 NeuronCores (B_local=1024).
Each core runs one fused matmul z.T = [Wi;Wh].T^T @ [x;h].T with K=8192 in
transposed orientation (partition dim = hidden) so the per-partition gate
biases ride the ScalarE activation's bias operand.  Matmuls in bf16
(full PE rate), accumulation + gate math in fp32; c/out are bf16 at the
DRAM boundary (error budget is ample).

Perf-critical structure (HW-measured on trn2):
- X resident in SBUF ([128, 64, 1024] bf16), loaded via 8 chunked DMAs on
  the SP HWDGE ring so the first matmuls start ~12us in.
- Weights streamed on the *Activation* HWDGE ring (own FIFO ring, half-block
  chunks, 6-deep pool) -- sharing a ring with the X stream or epilogue
  serializes the PE behind DMA waits (costs ~45%).
- Epilogue DMAs (c in, out/c_new out) on SWDGE (gpsimd) -- a third,
  independent DMA path.
- PSUM: 4 generations x 2 banks = all 8 banks, so the tensor engine streams
  accumulation groups back-to-back while ScalarE/DVE drain older groups.
"""

import sys

if "/opt/trn_rl_repo" not in sys.path:
    sys.path.insert(0, "/opt/trn_rl_repo")

import numpy as np
import ml_dtypes

import concourse.bass as bass
import concourse.mybir as mybir
from concourse import bacc
from concourse.tile import TileContext
from concourse.bass_utils import run_bass_kernel_spmd

B, IN, H = 8192, 4096, 4096
NCORES = 8
BL = B // NCORES          # 1024 batch rows per core
K = IN + H                # 8192 contraction
KS = K // 128             # 64 k-stripes
MBLK = H // 128           # 32 output-partition blocks
NB = BL // 512            # 2 psum tiles of 512 per m-block

BF16 = mybir.dt.bfloat16
F32 = mybir.dt.float32
AF = mybir.ActivationFunctionType

_cache = {}


def _build_nc(reps=1, wbufs=6, wsplit=2, xchunks=8):
    nc = bacc.Bacc("TRN2", target_bir_lowering=False)

    xh = nc.dram_tensor("xh2", [128, KS * BL], BF16, kind="ExternalInput")
    w = nc.dram_tensor("w", [MBLK, 128, KS * 128], BF16, kind="ExternalInput")
    bias = nc.dram_tensor("bias", [128, MBLK], F32, kind="ExternalInput")
    ct = nc.dram_tensor("ct2", [MBLK, 128, BL], BF16, kind="ExternalInput")
    outT = nc.dram_tensor("outP", [MBLK, 128, BL], BF16, kind="ExternalOutput")
    cnewT = nc.dram_tensor("cnewP", [MBLK, 128, BL], BF16, kind="ExternalOutput")

    KSC = KS // wsplit

    with TileContext(nc) as tc:
        with (
            tc.tile_pool(name="xpool", bufs=1) as xpool,
            tc.tile_pool(name="wpool", bufs=wbufs) as wpool,
            tc.tile_pool(name="bpool", bufs=1) as bpool,
            tc.tile_pool(name="cpool", bufs=2) as cpool,
            tc.tile_pool(name="spool", bufs=2) as spool,
            tc.tile_pool(name="gpool", bufs=2) as gpool,
            tc.tile_pool(name="cnpool", bufs=2) as cnpool,
            tc.tile_pool(name="upool", bufs=2) as upool,
            tc.tile_pool(name="opool", bufs=2) as opool,
            tc.tile_pool(name="psum", bufs=4, space="PSUM") as psum_pool,
        ):
            # bias rides SWDGE: it is first needed ~40us in, and keeping it off
            # the SP ring lets X chunk 0 start immediately.
            bias_sb = bpool.tile([128, MBLK], F32)
            nc.gpsimd.dma_start(out=bias_sb[:], in_=bias[:])

            for rep in range(reps):
                # X chunks alternate between the SP ring and SWDGE so two are
                # in flight during the startup ramp (m=0 consumes a chunk
                # every ~7us; a single ring delivers one every ~12us).
                X_sb = xpool.tile([128, KS, BL], BF16)
                xv = xh[:].rearrange("p (ks b) -> p ks b", ks=KS)
                KC = KS // xchunks
                for i, c0 in enumerate(range(0, KS, KC)):
                    eng = nc.sync if i % 2 == 0 else nc.gpsimd
                    eng.dma_start(out=X_sb[:, c0:c0 + KC, :], in_=xv[:, c0:c0 + KC, :])

                for m in range(MBLK):
                    chunks = []
                    for s in range(wsplit):
                        w_sb = wpool.tile([128, KSC * 128], BF16)
                        nc.scalar.dma_start(
                            out=w_sb[:],
                            in_=w[m, :, s * KSC * 128:(s + 1) * KSC * 128])
                        chunks.append(w_sb)

                    ps = [psum_pool.tile([128, 512], F32, name=f"ps{n}") for n in range(NB)]
                    for k in range(KS):
                        lhsT = chunks[k // KSC][:, (k % KSC) * 128:(k % KSC + 1) * 128]
                        for n in range(NB):
                            nc.tensor.matmul(
                                ps[n][:], lhsT,
                                X_sb[:, k, n * 512:(n + 1) * 512],
                                start=(k == 0), stop=(k == KS - 1))

                    c_t = cpool.tile([128, BL], BF16)
                    nc.gpsimd.dma_start(out=c_t[:], in_=ct[m])
                    cn_t = cnpool.tile([128, BL], BF16)
                    o_t = opool.tile([128, BL], BF16)
                    bvec = bias_sb[:, m:m + 1]
                    for n in range(NB):
                        cs = slice(n * 512, (n + 1) * 512)
                        s_t = spool.tile([128, 512], F32)
                        g_t = gpool.tile([128, 512], F32)
                        nc.scalar.activation(s_t[:], ps[n][:], AF.Sigmoid, bias=bvec)
                        nc.scalar.activation(g_t[:], ps[n][:], AF.Tanh, bias=bvec)
                        nc.vector.tensor_add(g_t[:], g_t[:], c_t[:, cs])   # g = c + g
                        nc.vector.tensor_mul(cn_t[:, cs], g_t[:], s_t[:])  # c_new = s*(c+g)
                        u_t = upool.tile([128, 512], F32)
                        nc.scalar.activation(u_t[:], cn_t[:, cs], AF.Tanh)
                        nc.vector.tensor_mul(o_t[:, cs], u_t[:], s_t[:])   # out = s*tanh
                    nc.gpsimd.dma_start(out=cnewT[m], in_=cn_t[:])
                    nc.gpsimd.dma_start(out=outT[m], in_=o_t[:])

    nc.finalize()
    return nc


def _prep_inputs(x, h, c, Wi, bi, Wh, bh):
    bf = ml_dtypes.bfloat16
    x = np.asarray(x, np.float32)
    h = np.asarray(h, np.float32)
    c = np.asarray(c, np.float32)

    # [x;h].T in bf16, retiled so each partition's stripe data is contiguous
    xhT = np.empty((K, B), dtype=bf)
    xhT[:IN] = x.T
    xhT[IN:] = h.T

    WT = np.empty((K, H), dtype=np.float32)
    WT[:IN] = np.asarray(Wi, np.float32).T
    WT[IN:] = np.asarray(Wh, np.float32).T
    # Wre[m, p, k*128+j] = WT[k*128+p, m*128+j]: each [128, 8192] m-slice has
    # 16KB contiguous per partition.
    Wre = np.ascontiguousarray(
        WT.reshape(KS, 128, MBLK, 128).transpose(2, 1, 0, 3).reshape(MBLK, 128, KS * 128)
    ).astype(bf)

    bias_re = np.ascontiguousarray(
        (np.asarray(bi, np.float32) + np.asarray(bh, np.float32)).reshape(MBLK, 128).T
    )

    cT = c.T  # [H, B]
    in_maps = []
    for cid in range(NCORES):
        bs = slice(cid * BL, (cid + 1) * BL)
        xh_s = np.ascontiguousarray(xhT[:, bs])
        xh2 = np.ascontiguousarray(
            xh_s.reshape(KS, 128, BL).transpose(1, 0, 2).reshape(128, KS * BL))
        ct2 = np.ascontiguousarray(cT[:, bs].reshape(MBLK, 128, BL)).astype(bf)
        in_maps.append({"xh2": xh2, "w": Wre, "bias": bias_re, "ct2": ct2})
    return in_maps


def _unpack(res_list):
    outs, cns = [], []
    for r in res_list:
        outs.append(np.asarray(r["outP"], np.float32).reshape(H, BL))
        cns.append(np.asarray(r["cnewP"], np.float32).reshape(H, BL))
    outT = np.concatenate(outs, axis=1)   # [H, B]
    cnewT = np.concatenate(cns, axis=1)
    return (np.ascontiguousarray(outT.T), np.ascontiguousarray(cnewT.T))


def kernel(x, h, c, Wi, bi, Wh, bh):
    if "nc" not in _cache:
        _cache["nc"] = _build_nc()
    nc = _cache["nc"]

    in_maps = _prep_inputs(x, h, c, Wi, bi, Wh, bh)
    res = run_bass_kernel_spmd(nc, in_maps, core_ids=list(range(NCORES)))
    return _unpack(res.results)


# revision 4
# speedup vs baseline: 1.0153x; 1.0153x over previous
"""LSTM-cell (shared-gate) Trainium2 kernel.

Reference computes, for B=8192, IN=H=4096:
    z = x @ Wi.T + bi + h @ Wh.T + bh        # [B, H]
    s = sigmoid(z); g = tanh(z)
    c_new = c*s + s*g = s*(c+g)
    out = s*tanh(c_new)
    returns (out, c_new)

Strategy: data-parallel over batch across 8 NeuronCores (B_local=1024).
Each core runs one fused matmul z.T = [Wi;Wh].T^T @ [x;h].T with K=8192 in
transposed orientation (partition dim = hidden) so the per-partition gate
biases ride the ScalarE activation's bias operand.  Matmuls in bf16
(full PE rate), accumulation + gate math in fp32; c/out are bf16 at the
DRAM boundary (error budget is ample).

Perf-critical structure (HW-measured on trn2):
- X resident in SBUF ([128, 64, 1024] bf16), loaded via 8 chunked DMAs on
  the SP HWDGE ring so the first matmuls start ~12us in.
- Weights streamed on the *Activation* HWDGE ring (own FIFO ring, half-block
  chunks, 6-deep pool) -- sharing a ring with the X stream or epilogue
  serializes the PE behind DMA waits (costs ~45%).
- Epilogue DMAs (c in, out/c_new out) on SWDGE (gpsimd) -- a third,
  independent DMA path.
- PSUM: 4 generations x 2 banks = all 8 banks, so the tensor engine streams
  accumulation groups back-to-back while ScalarE/DVE drain older groups.
"""

import sys

if "/opt/trn_rl_repo" not in sys.path:
    sys.path.insert(0, "/opt/trn_rl_repo")

import numpy as np
import ml_dtypes

import concourse.bass as bass
import concourse.mybir as mybir
from concourse import bacc
from concourse.tile import TileContext
from concourse.bass_utils import run_bass_kernel_spmd

B, IN, H = 8192, 4096, 4096
NCORES = 8
BL = B // NCORES          # 1024 batch rows per core
K = IN + H                # 8192 contraction
KS = K // 128             # 64 k-stripes
MBLK = H // 128           # 32 output-partition blocks
NB = BL // 512            # 2 psum tiles of 512 per m-block

BF16 = mybir.dt.bfloat16
F32 = mybir.dt.float32
AF = mybir.ActivationFunctionType

_cache = {}


def _build_nc(reps=1, wbufs=6, wsplit=2, xchunks=8):
    nc = bacc.Bacc("TRN2", target_bir_lowering=False)

    xh = nc.dram_tensor("xh2", [128, KS * BL], BF16, kind="ExternalInput")
    w = nc.dram_tensor("w", [MBLK, 128, KS * 128], BF16, kind="ExternalInput")
    bias = nc.dram_tensor("bias", [128, MBLK], F32, kind="ExternalInput")
    ct = nc.dram_tensor("ct2", [MBLK, 128, BL], BF16, kind="ExternalInput")
    outT = nc.dram_tensor("outP", [MBLK, 128, BL], BF16, kind="ExternalOutput")
    cnewT = nc.dram_tensor("cnewP", [MBLK, 128, BL], BF16, kind="ExternalOutput")

    KSC = KS // wsplit

    with TileContext(nc) as tc:
        with (
            tc.tile_pool(name="xpool", bufs=1) as xpool,
            tc.tile_pool(name="wpool", bufs=wbufs) as wpool,
            tc.tile_pool(name="bpool", bufs=1) as bpool,
            tc.tile_pool(name="cpool", bufs=2) as cpool,
            tc.tile_pool(name="spool", bufs=2) as spool,
            tc.tile_pool(name="gpool", bufs=2) as gpool,
            tc.tile_pool(name="cnpool", bufs=2) as cnpool,
            tc.tile_pool(name="upool", bufs=2) as upool,
            tc.tile_pool(name="opool", bufs=2) as opool,
            tc.tile_pool(name="psum", bufs=4, space="PSUM") as psum_pool,
        ):
            # bias rides SWDGE: it is first needed ~40us in, and keeping it off
            # the SP ring lets X chunk 0 start immediately.
            bias_sb = bpool.tile([128, MBLK], F32)
            nc.gpsimd.dma_start(out=bias_sb[:], in_=bias[:])

            for rep in range(reps):
                # X chunks all on the otherwise-idle SP ring (splitting them
                # onto SWDGE measured ~100us worse: contends with epilogue).
                X_sb = xpool.tile([128, KS, BL], BF16)
                xv = xh[:].rearrange("p (ks b) -> p ks b", ks=KS)
                KC = KS // xchunks
                for c0 in range(0, KS, KC):
                    nc.sync.dma_start(out=X_sb[:, c0:c0 + KC, :], in_=xv[:, c0:c0 + KC, :])

                for m in range(MBLK):
                    chunks = []
                    for s in range(wsplit):
                        w_sb = wpool.tile([128, KSC * 128], BF16)
                        nc.scalar.dma_start(
                            out=w_sb[:],
                            in_=w[m, :, s * KSC * 128:(s + 1) * KSC * 128])
                        chunks.append(w_sb)

                    ps = [psum_pool.tile([128, 512], F32, name=f"ps{n}") for n in range(NB)]
                    for k in range(KS):
                        lhsT = chunks[k // KSC][:, (k % KSC) * 128:(k % KSC + 1) * 128]
                        for n in range(NB):
                            nc.tensor.matmul(
                                ps[n][:], lhsT,
                                X_sb[:, k, n * 512:(n + 1) * 512],
                                start=(k == 0), stop=(k == KS - 1))

                    c_t = cpool.tile([128, BL], BF16)
                    nc.gpsimd.dma_start(out=c_t[:], in_=ct[m])
                    cn_t = cnpool.tile([128, BL], BF16)
                    o_t = opool.tile([128, BL], BF16)
                    bvec = bias_sb[:, m:m + 1]
                    for n in range(NB):
                        cs = slice(n * 512, (n + 1) * 512)
                        s_t = spool.tile([128, 512], F32)
                        g_t = gpool.tile([128, 512], F32)
                        nc.scalar.activation(s_t[:], ps[n][:], AF.Sigmoid, bias=bvec)
                        nc.scalar.activation(g_t[:], ps[n][:], AF.Tanh, bias=bvec)
                        nc.vector.tensor_add(g_t[:], g_t[:], c_t[:, cs])   # g = c + g
                        nc.vector.tensor_mul(cn_t[:, cs], g_t[:], s_t[:])  # c_new = s*(c+g)
                        u_t = upool.tile([128, 512], F32)
                        nc.scalar.activation(u_t[:], cn_t[:, cs], AF.Tanh)
                        nc.vector.tensor_mul(o_t[:, cs], u_t[:], s_t[:])   # out = s*tanh
                    nc.gpsimd.dma_start(out=cnewT[m], in_=cn_t[:])
                    nc.gpsimd.dma_start(out=outT[m], in_=o_t[:])

    nc.finalize()
    return nc


def _prep_inputs(x, h, c, Wi, bi, Wh, bh):
    bf = ml_dtypes.bfloat16
    x = np.asarray(x, np.float32)
    h = np.asarray(h, np.float32)
    c = np.asarray(c, np.float32)

    # [x;h].T in bf16, retiled so each partition's stripe data is contiguous
    xhT = np.empty((K, B), dtype=bf)
    xhT[:IN] = x.T
    xhT[IN:] = h.T

    WT = np.empty((K, H), dtype=np.float32)
    WT[:IN] = np.asarray(Wi, np.float32).T
    WT[IN:] = np.asarray(Wh, np.float32).T
    # Wre[m, p, k*128+j] = WT[k*128+p, m*128+j]: each [128, 8192] m-slice has
    # 16KB contiguous per partition.
    Wre = np.ascontiguousarray(
        WT.reshape(KS, 128, MBLK, 128).transpose(2, 1, 0, 3).reshape(MBLK, 128, KS * 128)
    ).astype(bf)

    bias_re = np.ascontiguousarray(
        (np.asarray(bi, np.float32) + np.asarray(bh, np.float32)).reshape(MBLK, 128).T
    )

    cT = c.T  # [H, B]
    in_maps = []
    for cid in range(NCORES):
        bs = slice(cid * BL, (cid + 1) * BL)
        xh_s = np.ascontiguousarray(xhT[:, bs])
        xh2 = np.ascontiguousarray(
            xh_s.reshape(KS, 128, BL).transpose(1, 0, 2).reshape(128, KS * BL))
        ct2 = np.ascontiguousarray(cT[:, bs].reshape(MBLK, 128, BL)).astype(bf)
        in_maps.append({"xh2": xh2, "w": Wre, "bias": bias_re, "ct2": ct2})
    return in_maps


def _unpack(res_list):
    outs, cns = [], []
    for r in res_list:
        outs.append(np.asarray(r["outP"], np.float32).reshape(H, BL))
        cns.append(np.asarray(r["cnewP"], np.float32).reshape(H, BL))
    outT = np.concatenate(outs, axis=1)   # [H, B]
    cnewT = np.concatenate(cns, axis=1)
    return (np.ascontiguousarray(outT.T), np.ascontiguousarray(cnewT.T))


def kernel(x, h, c, Wi, bi, Wh, bh):
    if "nc" not in _cache:
        _cache["nc"] = _build_nc()
    nc = _cache["nc"]

    in_maps = _prep_inputs(x, h, c, Wi, bi, Wh, bh)
    res = run_bass_kernel_spmd(nc, in_maps, core_ids=list(range(NCORES)))
    return _unpack(res.results)
